# revision 53
# baseline (speedup 1.0000x reference)
"""Trainium2 Bass kernel for a 2-layer message-passing GNN (BaselineGNN).

Reference computation (N=4096 nodes, IN=512, HID=4096, E=65536 edges):
    h   = x @ We.T + be                                   [N, HID]
    for W, b in ((W1, b1), (W2, b2)):
        aggr = segment_sum(h[col], row)                   [N, HID]
        h    = relu(aggr @ W.T + b)
    hm  = mean(h, axis=1)                                 [N]
    z   = relu(hm @ Wc1.T + bc1)                          [HID//2]
    out = (z @ Wc2.T + bc2).squeeze(-1)                   scalar

Strategy (8 NeuronCores, node-parallel):
  * segment_sum == A @ h with A the [N, N] adjacency-count matrix (0.4%
    dense).  A's entries are small integer counts -> exactly representable
    in bf16, so aggregation runs as a dense TensorEngine matmul instead of
    a DMA gather/scatter (less HBM traffic, PE does the work).
  * Nodes are sharded: core c owns rows 512c..512c+512.  Weight matmuls
    are then fully local; each core computes A_c @ h with A_c = A[rows_c].
  * The embed layer is folded into layer-1 aggregation:
        aggr1_c = (A_c @ x_ext) @ We_ext.T
    where x_ext carries an extra all-ones column and We_ext.T an extra
    b_embed row, so the (degree-weighted) embed bias is exact.
  * One 4MB AllGather shares h1 between the layers; a tiny AllGather
    shares hm for the (classifier-sharded) head.  Each core emits a
    partial classifier dot product; the host sums them (+ bc2).
  * All big matmuls run in bf16 with fp32 PSUM accumulation; the
    classifier head runs in fp32.

Everything is expressed "transposed-free": every matmul is
mxn = kxm.T @ kxn with operands stored so no transposes are ever needed.
"""

import contextlib
from dataclasses import replace as dc_replace

import numpy as np
import ml_dtypes

import concourse.bass as bass
import concourse.mybir as mybir
import concourse.tile as tile
from concourse import bacc
from concourse.bass_interp import get_hw_module
from concourse.bass_utils import run_bass_kernel_spmd
from concourse.kernels.tile_matmul import (
    composable_matmul_tile_kernel,
    dma_from_dram_kxm,
    dma_from_dram_kxn,
    dma_to_dram_mxn,
    k_pool_min_bufs,
    matmul_tile_kernel,
    scalar_copyback,
)

N = 4096          # nodes
IN_DIM = 512
HID = 4096
NCORES = 8
S = N // NCORES           # nodes per core (512)
KE = 640                  # extended embed contraction (512 + 1 ones col, padded to 5*128)
CHID = HID // 2           # classifier hidden (2048)


BF16 = mybir.dt.bfloat16
F32 = mybir.dt.float32
FP8 = mybir.dt.float8e4

# fp8 (DoubleRow) for the three large matmuls: the adjacency counts are
# exact in e4m3, activations are O(1)-O(100) (well inside e4m3 range), and
# the 0.02-scale weights are stored pre-scaled by WSCALE so they land in
# e4m3's normal range; the eviction multiplies by 1/WSCALE.
USE_FP8 = True
WSCALE = 64.0

_COMPILED = None


def _bias_relu_reducer(nc_b, bias_sb, accum=None, scale=None):
    """PSUM->SBUF evict: sbuf = relu(psum*scale + bias[n_slice]).

    bias_sb: [128, N_total] SBUF tile, bias broadcast across partitions.
    accum:   optional [128, M_SUBTILES, N_TILES] fp32 tile; when given, the
             relu also row-sum-accumulates into accum[:, m_subtile, n_tile].
    """

    def _reducer(nc, psum, sbuf, md):
        start = md.n_tile_idx * md.n_tile + md.n_subtile_idx * md.n_subtile
        if scale is not None:
            nc.vector.tensor_scalar_mul(
                sbuf[:, :, : md.n_slice_size], psum[:, : md.n_slice_size], scale
            )
            nc.vector.tensor_add(
                out=sbuf[:, :, : md.n_slice_size],
                in0=sbuf[:, :, : md.n_slice_size],
                in1=bias_sb[:, start : start + md.n_slice_size],
            )
        else:
            nc.vector.tensor_add(
                out=sbuf[:, :, : md.n_slice_size],
                in0=psum[:, : md.n_slice_size],
                in1=bias_sb[:, start : start + md.n_slice_size],
            )
        if accum is None:
            nc.vector.tensor_scalar_max(sbuf, sbuf, 0.0)
        else:
            ni = md.n_tile_idx * md.n_subtiles + md.n_subtile_idx
            nc.scalar.activation(
                out=sbuf,
                in_=sbuf,
                func=mybir.ActivationFunctionType.Relu,
                accum_out=accum[:, md.m_subtile_idx, ni : ni + 1],
            )

    return _reducer


def _matmul_custom(ctx, tc, kxm_ap, kxn_ap, reducer, consumer, output_type=F32,
                   psum_n_bufs=2, kxm_cache=None, kxn_cache=None, producer=None):
    """composable matmul with custom psum reducer / mxn consumer / SBUF caches.

    kxm_cache / kxn_cache: [128, K//128, M-or-N] SBUF tiles already holding
    the operand (no DMA is issued for that side).  producer: optional
    mxn_subtile_producer returning the SBUF tile the reducer writes (used to
    write results directly into a persistent SBUF cache).
    """
    nc = tc.nc
    num_bufs = k_pool_min_bufs(kxn_ap)
    if kxm_cache is None:
        kxm_pool = ctx.enter_context(tc.tile_pool(name="kxm_pool", bufs=num_bufs))
    else:
        kxm_pool = None
    if kxn_cache is None:
        kxn_pool = ctx.enter_context(tc.tile_pool(name="kxn_pool", bufs=num_bufs))
    else:
        kxn_pool = None
    kxm_producer, kxm_shape = dma_from_dram_kxm(kxm_pool, kxm_ap, kxm_cache=kxm_cache)
    kxn_producer, kxn_shape = dma_from_dram_kxn(kxn_pool, kxn_ap, kxn_cache=kxn_cache)
    composable_matmul_tile_kernel(
        tc=tc,
        kxm_shape=kxm_shape,
        kxn_shape=kxn_shape,
        output_type=output_type if producer is None else None,
        kxm_producer=kxm_producer,
        kxn_producer=kxn_producer,
        mxn_subtile_reducer=reducer,
        mxn_consumer=consumer,
        mxn_subtile_producer=producer,
        psum_n_bufs=psum_n_bufs,
    )


def _build_graph():
    nc = bacc.Bacc(
        "TRN2",
        target_bir_lowering=False,
        debug=False,
        enable_asserts=False,
        num_devices=NCORES,
    )

    ADT = FP8 if USE_FP8 else BF16   # dtype of layer-2 aggregation operands
    WDT = FP8 if USE_FP8 else BF16   # dtype of W1/W2

    # ---- kernel I/O (per core) ----
    xe = nc.dram_tensor("xe", [N, KE], ADT, kind="ExternalInput")         # x_ext (replicated)
    at8 = nc.dram_tensor("at8", [N, S], ADT, kind="ExternalInput")        # A.T[:, rows_c] (sharded)
    wcb = nc.dram_tensor("wcb", [KE, HID], BF16, kind="ExternalInput")    # We_ext.T @ W1.T (replicated)
    w2 = nc.dram_tensor("w2", [HID, HID], WDT, kind="ExternalInput")      # W2.T * WSCALE (replicated)
    b1 = nc.dram_tensor("b1", [128, HID], F32, kind="ExternalInput")      # b1 bcast (replicated)
    b2 = nc.dram_tensor("b2", [128, HID], F32, kind="ExternalInput")      # b2 bcast (replicated)
    wc1 = nc.dram_tensor("wc1", [S, CHID], F32, kind="ExternalInput")     # Wc1.T row-chunk (sharded)
    bc1 = nc.dram_tensor("bc1", [128, CHID // 128], F32, kind="ExternalInput")  # bc1 [128,16]
    wc2 = nc.dram_tensor("wc2", [128, CHID // 128], F32, kind="ExternalInput")  # Wc2 [128,16]
    res = nc.dram_tensor("res", [1, 1], F32, kind="ExternalOutput")       # final scalar (pre-bc2)

    # ---- internal DRAM ----
    # h1 is split into column quarters so each AllGather overlaps with the
    # production of the next quarter (M3) / consumption of the previous
    # one (M4).
    NAG = 4
    HQ = HID // NAG
    h1c_q = [nc.dram_tensor(f"h1c{i}", [S, HQ], ADT) for i in range(NAG)]
    h1f_q = [
        nc.dram_tensor(f"h1f{i}", [N, HQ], ADT, addr_space="Shared")
        for i in range(NAG)
    ]
    zb = nc.dram_tensor("zb", [1, CHID], F32)           # local partial Wc1 @ hm
    zf = nc.dram_tensor("zf", [1, CHID], F32, addr_space="Shared")  # allreduced

    MSUB = S // 128   # 4 m-subtiles in a 512-row tile
    NT = HID // 512   # 8 n-tiles of 512

    with tile.TileContext(nc) as tc:
        with contextlib.ExitStack() as octx:
            const = octx.enter_context(tc.tile_pool(name="const", bufs=1))
            b1_sb = const.tile([128, HID], F32, name="b1_sb")
            b2_sb = const.tile([128, HID], F32, name="b2_sb")
            hm_parts = const.tile([128, MSUB, NT], F32, name="hm_parts")
            hm_sb = const.tile([128, MSUB], F32, name="hm_sb")
            nc.any.memset(hm_parts[:], 0.0)

            head = octx.enter_context(tc.tile_pool(name="head", bufs=1))
            CI = CHID // 128  # 16
            wc1_t = head.tile([128, MSUB, CHID], F32, name="wc1_t")
            bc1_t = head.tile([128, CI], F32, name="bc1_t")
            wc2_t = head.tile([128, CI], F32, name="wc2_t")
            zp_t = head.tile([1, CHID], F32, name="zp_t")
            z2_t = head.tile([128, CI], F32, name="z2_t")
            zcol_t = head.tile([128, 1], F32, name="zcol_t")
            ones_t = head.tile([128, 1], F32, name="ones_t")
            r_t = head.tile([1, 1], F32, name="r_t")
            nc.any.memset(ones_t[:], 1.0)

            # persistent SBUF caches for chained-matmul intermediates: the
            # producing matmul's reducer writes straight into the cache and
            # the consuming matmul reads it as kxm/kxn — no DRAM roundtrip
            cache = octx.enter_context(tc.tile_pool(name="cache", bufs=1))
            at8_c = cache.tile([128, N // 128, S], ADT, name="at8_c")
            tT_c = cache.tile([128, KE // 128, S], BF16, name="tT_c")
            a2T_c = cache.tile([128, HID // 128, S], ADT, name="a2T_c")
            # chunked so M1's first k-tile doesn't wait on the whole 2MB
            for po in range(0, N // 128, 4):
                nc.sync.dma_start(
                    out=at8_c[:, po : po + 4, :],
                    in_=at8[:, :].rearrange("(po pi) n -> pi po n", pi=128)[
                        :, po : po + 4, :
                    ],
                )
            noop = lambda nc_, sbuf, md: None

            # M1: tT = (A_c @ x_ext).T = xe.T @ A_c.T        [KE, S]
            with contextlib.ExitStack() as ctx:
                _matmul_custom(
                    ctx, tc, xe[:, :], at8_c[:, :, :],
                    scalar_copyback(), noop,
                    kxn_cache=at8_c[:, :, :],
                    producer=lambda nc_, md: tT_c[:, md.m_tile_idx : md.m_tile_idx + 1, :],
                )
            # b1 prefetch after M1 so it doesn't starve M1's tiles in the
            # DMA queues (scheduler priority = trace order)
            nc.sync.dma_start(out=b1_sb[:, :], in_=b1[:, :])

            # M3: h1_c = relu(tT.T @ (We_ext.T W1.T) + b1)   [S, HID]
            # (layer-1 transform is low-rank: contraction is only KE=640)
            # consumer routes each column quarter to its own AG input tensor
            cons_q = [dma_to_dram_mxn(h1c_q[i][:, :]) for i in range(NAG)]
            NTQ = NT // NAG  # n-tiles per quarter

            def h1_consumer(nc_, sbuf, md):
                q = md.n_tile_idx // NTQ
                cons_q[q](nc_, sbuf, dc_replace(md, n_tile_idx=md.n_tile_idx % NTQ))

            wsc = (1.0 / WSCALE) if USE_FP8 else None
            with contextlib.ExitStack() as ctx:
                _matmul_custom(
                    ctx, tc, tT_c[:, :, :], wcb[:, :],
                    _bias_relu_reducer(nc, b1_sb),
                    h1_consumer,
                    output_type=ADT,
                    kxm_cache=tT_c[:, :, :],
                )
            # AllGather h1 quarters; M4 on quarter i overlaps later AGs
            for i in range(NAG):
                nc.gpsimd.collective_compute(
                    "AllGather",
                    mybir.AluOpType.bypass,
                    ins=[h1c_q[i][:, :].opt()],
                    outs=[h1f_q[i][:, :].opt()],
                    replica_groups=[list(range(NCORES))],
                )
            # remaining prefetches (needed by M5's reducer and M6) go after
            # the AG triggers so the h1 bounce writes aren't queued behind them
            nc.sync.dma_start(out=b2_sb[:, :], in_=b2[:, :])
            nc.sync.dma_start(
                out=wc1_t[:, :, :],
                in_=wc1[:, :].rearrange("(po pi) n -> pi po n", pi=128),
            )
            nc.sync.dma_start(out=bc1_t[:, :], in_=bc1[:, :])
            nc.sync.dma_start(out=wc2_t[:, :], in_=wc2[:, :])

            # M4: a2T = (A_c @ h1).T = h1f.T @ A_c.T         [HID, S]
            QPO = HQ // 128  # 8 cache k-slots per quarter
            for i in range(NAG):
                with contextlib.ExitStack() as ctx:
                    _matmul_custom(
                        ctx, tc, h1f_q[i][:, :], at8_c[:, :, :],
                        scalar_copyback(), noop,
                        kxn_cache=at8_c[:, :, :],
                        producer=lambda nc_, md, i=i: a2T_c[
                            :,
                            QPO * i + 4 * md.m_tile_idx : QPO * i + 4 * md.m_tile_idx + 4,
                            :,
                        ],
                    )
            # M5: h2_c = relu(a2T.T @ w2 + b2); row-sums stream into hm_parts
            with contextlib.ExitStack() as ctx:
                _matmul_custom(
                    ctx, tc, a2T_c[:, :, :], w2[:, :],
                    _bias_relu_reducer(nc, b2_sb, accum=hm_parts, scale=wsc),
                    noop,
                    output_type=F32,
                    kxm_cache=a2T_c[:, :, :],
                )
            # hm_c = rowsum(h2_c) / HID (local nodes only, kept in SBUF)
            nc.vector.tensor_reduce(
                out=hm_sb[:, :], in_=hm_parts[:, :, :],
                axis=mybir.AxisListType.X, op=mybir.AluOpType.add,
            )
            nc.vector.tensor_scalar_mul(hm_sb[:, :], hm_sb[:, :], 1.0 / HID)

            # M6 (head, fp32): zp = Wc1[:, local] @ hm_local, AllReduce over
            # cores, then z = relu(z + bc1), res = z . Wc2
            hpsum = octx.enter_context(tc.tile_pool(name="hpsum", bufs=1, space="PSUM"))
            NB = CHID // 512  # 4 psum banks
            ps = [hpsum.tile([128, 512], F32, name=f"ps{j}") for j in range(NB)]
            psr = hpsum.tile([128, 512], F32, name="psr")
            for j in range(NB):
                for ko in range(MSUB):
                    nc.tensor.matmul(
                        ps[j][0:1, :],
                        hm_sb[:, ko : ko + 1],
                        wc1_t[:, ko, 512 * j : 512 * (j + 1)],
                        start=(ko == 0),
                        stop=(ko == MSUB - 1),
                    )
                nc.vector.tensor_copy(
                    out=zp_t[:, 512 * j : 512 * (j + 1)], in_=ps[j][0:1, :]
                )
            nc.sync.dma_start(out=zb[:, :], in_=zp_t[:, :])
            nc.gpsimd.collective_compute(
                "AllReduce",
                mybir.AluOpType.add,
                ins=[zb[:, :].opt()],
                outs=[zf[:, :].opt()],
                replica_groups=[list(range(NCORES))],
            )
            # epilogue on z viewed as [128, 16] so the DVE ops use all lanes
            nc.sync.dma_start(
                out=z2_t[:, :], in_=zf[:, :].rearrange("o (p i) -> p (o i)", p=128)
            )
            nc.vector.tensor_add(out=z2_t[:, :], in0=z2_t[:, :], in1=bc1_t[:, :])
            nc.vector.tensor_scalar_max(z2_t[:, :], z2_t[:, :], 0.0)
            nc.vector.tensor_mul(out=z2_t[:, :], in0=z2_t[:, :], in1=wc2_t[:, :])
            nc.vector.tensor_reduce(
                out=zcol_t[:, :], in_=z2_t[:, :],
                axis=mybir.AxisListType.X, op=mybir.AluOpType.add,
            )
            # cross-partition sum via a 128x1 ones matmul
            nc.tensor.matmul(
                psr[0:1, 0:1], ones_t[:, 0:1], zcol_t[:, 0:1], start=True, stop=True
            )
            nc.vector.tensor_copy(out=r_t[:, :], in_=psr[0:1, 0:1])
            nc.sync.dma_start(out=res[:, :], in_=r_t[:, :])

    nc.compile()
    nc.m = get_hw_module(nc.m)
    return nc


def get_compiled():
    global _COMPILED
    if _COMPILED is None:
        _COMPILED = _build_graph()
    return _COMPILED


def _bf16(a):
    return np.ascontiguousarray(np.asarray(a, dtype=np.float32)).astype(ml_dtypes.bfloat16)


def _f32(a):
    return np.ascontiguousarray(np.asarray(a, dtype=np.float32))


_NP_FP8 = mybir.dt.np(FP8)


def _adt(a):
    """Convert to the layer-2 aggregation dtype (fp8 or bf16)."""
    a = np.ascontiguousarray(np.asarray(a, dtype=np.float32))
    return a.astype(_NP_FP8 if USE_FP8 else ml_dtypes.bfloat16)


def make_in_maps(x, edge_index, W_embed, b_embed, W1, b1, W2, b2, Wc1, bc1, Wc2, bc2):
    x = _f32(x)
    ei = np.asarray(edge_index).astype(np.int64)
    # adjacency counts, transposed: AT[src, dst] = #edges src->dst
    counts = np.bincount(ei[1] * N + ei[0], minlength=N * N).astype(np.float32)
    AT = counts.reshape(N, N)

    x_ext = np.zeros((N, KE), np.float32)
    x_ext[:, :IN_DIM] = x
    x_ext[:, IN_DIM] = 1.0

    we_ext = np.zeros((KE, HID), np.float32)
    we_ext[:IN_DIM] = _f32(W_embed).T
    we_ext[IN_DIM] = _f32(b_embed)
    # layer-1 transform is low-rank: fold We_ext.T @ W1.T on the host
    wcb_np = _bf16(we_ext @ _f32(W1).T)

    xe_np = _adt(x_ext)
    at8_np = _adt(AT)
    wmul = WSCALE if USE_FP8 else 1.0
    w2_np = _adt(_f32(W2).T * wmul) if USE_FP8 else _bf16(_f32(W2).T)
    b1_np = _f32(np.broadcast_to(_f32(b1), (128, HID)))
    b2_np = _f32(np.broadcast_to(_f32(b2), (128, HID)))
    wc1T = _f32(Wc1).T  # [HID(nodes), CHID]
    wc2_row = _f32(Wc2).reshape(128, CHID // 128)
    bc1_full = _f32(bc1).reshape(128, CHID // 128)

    in_maps = []
    for c in range(NCORES):
        rows = slice(S * c, S * (c + 1))
        in_maps.append(
            {
                "xe": xe_np,
                "wcb": wcb_np,
                "at8": np.ascontiguousarray(at8_np[:, rows]),
                "w2": w2_np,
                "b1": b1_np,
                "b2": b2_np,
                "wc1": np.ascontiguousarray(wc1T[rows, :]),
                "bc1": bc1_full,
                "wc2": wc2_row,
            }
        )
    return in_maps


def kernel(**inputs):
    nc = get_compiled()
    in_maps = make_in_maps(**inputs)
    bres = run_bass_kernel_spmd(nc, in_maps, core_ids=list(range(NCORES)))
    val = np.float32(bres.results[0]["res"][0, 0])
    bc2 = np.asarray(inputs["bc2"], dtype=np.float32).reshape(-1)
    out = np.asarray(val + bc2[0], dtype=np.float32).reshape(())
    return out
